# revision 40
# baseline (speedup 1.0000x reference)
"""GAT layer kernel for Trainium2, SPMD across 8 NeuronCores.

Math (per batch b):
    h[n]   = x[b,n] @ proj_w[n] + proj_b[n]
    s[i,j] = h[i] . a_src[j] + h[j] . a_dst[j]
    att    = softmax_j( mask(leaky_relu(s)) ),  mask: (0<dist<0.5)|eye
    y[i]   = sum_j att[i,j] h[j]

Sharding: destination rows i are split into 8 blocks of 512 per core.
Each core projects its own 512 nodes, the projected features are
all-gathered (bf16), and each core computes its row block of
scores/softmax/aggregation flash-style (scores stay in PSUM/SBUF).

All DMA access patterns are contiguous: the host pre-transposes
dist (to [j, i] per block), a_src (to [c, j]), x (to [c2, pair, b])
and proj_b (to [c, n]) so the device never does element-granular
strided gathers. The adjacency mask enters the scores as a TensorE
accumulation (-1e15*I @ notE^T) so no DVE pass over the score matrix
is needed; LeakyReLU is split between ScalarE (Lrelu) and VectorE to
balance engine time; exp runs on ScalarE in bf16.
"""

import numpy as np
import ml_dtypes

BF16 = ml_dtypes.bfloat16

B = 4
N = 4096
C = 64
R = 8            # cores
NB = N // R      # 512 rows per core
JT = 128         # j-tile width (partition dim of transposed scores)
NJT = N // JT    # 32 j-tiles
GJ = 2           # j-tiles per PSUM/elementwise group
NG = NJT // GJ   # 16 groups
C1 = C + 1       # h + ones column
NP = NB // 2     # node pairs per core (256)
ALPHA = 0.01
NEG = -1.0e15

Z1 = NB * B * C1     # region 1 of gather payload: [n][b][h(64), 1.0]
Z2 = B * NB          # region 2: d[b][n]
ZT = Z1 + Z2

_CACHE = {}


def _build():
    import concourse.bass as bass
    import concourse.tile as tile
    from concourse import bacc, mybir
    from concourse.masks import make_identity

    f32 = mybir.dt.float32
    bf16 = mybir.dt.bfloat16
    Alu = mybir.AluOpType
    Act = mybir.ActivationFunctionType

    nc = bacc.Bacc("TRN2", target_bir_lowering=False, debug=False, num_devices=R)

    wb = nc.dram_tensor("wb", [2 * C, NP, 2 * C], bf16, kind="ExternalInput").ap()
    xq = nc.dram_tensor("xq", [2 * C, NP, B], bf16, kind="ExternalInput").ap()
    bbT = nc.dram_tensor("bbT", [C, NB], f32, kind="ExternalInput").ap()
    adT = nc.dram_tensor("adT", [NB, C], bf16, kind="ExternalInput").ap()
    dbT = nc.dram_tensor("dbT", [N, NB], f32, kind="ExternalInput").ap()
    asT = nc.dram_tensor("asT", [C, N], bf16, kind="ExternalInput").ap()
    out = nc.dram_tensor("out", [B, NB, C], f32, kind="ExternalOutput").ap()

    z_local = nc.dram_tensor("z_local", [ZT], bf16, kind="Internal")
    z_full = nc.dram_tensor("z_full", [R, ZT], bf16, kind="Internal",
                            addr_space="Shared")

    with tile.TileContext(nc) as tc:
        _body(tc, nc, bass, mybir, make_identity, f32, bf16, Alu, Act,
              wb, xq, bbT, adT, dbT, asT, out, z_local, z_full)
    nc.compile()
    return nc


def _body(tc, nc, bass, mybir, make_identity, f32, bf16, Alu, Act,
          wb, xq, bbT, adT, dbT, asT, out, z_local, z_full):
    from contextlib import ExitStack
    import os
    n_groups = int(os.environ.get("GAT_NGROUPS", "4"))
    n_batch = int(os.environ.get("GAT_NBATCH", str(B)))
    skip_main = os.environ.get("GAT_SKIP_MAIN", "0") == "1"
    skip_coll = os.environ.get("GAT_SKIP_COLL", "0") == "1"
    skip_vall = os.environ.get("GAT_SKIP_VALL", "0") == "1"
    skip_mask = os.environ.get("GAT_SKIP_MASK", "0") == "1"
    skip_phase_a = os.environ.get("GAT_SKIP_PHASE_A", "0") == "1"
    pa_level = int(os.environ.get("GAT_PA_LEVEL", "7"))
    d_nomask = os.environ.get("GAT_D_NOMASK", "0") == "1"
    d_noew = os.environ.get("GAT_D_NOEW", "0") == "1"
    d_noagg = os.environ.get("GAT_D_NOAGG", "0") == "1"
    d_noscores = os.environ.get("GAT_D_NOSCORES", "0") == "1"
    ew_split = int(os.environ.get("GAT_EW_SPLIT", "1024"))

    ctx = ExitStack()
    with ctx:
        const = ctx.enter_context(tc.tile_pool(name="const", bufs=1))
        pa_w = ctx.enter_context(tc.tile_pool(name="pa_w", bufs=8))
        pa_x = ctx.enter_context(tc.tile_pool(name="pa_x", bufs=8))
        pa_sb = ctx.enter_context(tc.tile_pool(name="pa_sb", bufs=2))
        mk_sb = ctx.enter_context(tc.tile_pool(name="mk_sb", bufs=3))
        mn_sb = ctx.enter_context(tc.tile_pool(name="mn_sb", bufs=2))
        ew_sb = ctx.enter_context(tc.tile_pool(name="ew_sb", bufs=7))
        ep_sb = ctx.enter_context(tc.tile_pool(name="ep_sb", bufs=2))
        ps_s = ctx.enter_context(tc.tile_pool(name="ps_s", bufs=2, space="PSUM"))
        ps_y = ctx.enter_context(tc.tile_pool(name="ps_y", bufs=2, space="PSUM"))
        ps_t = ctx.enter_context(tc.tile_pool(name="ps_t", bufs=2, space="PSUM"))

        min_mode = int(os.environ.get("GAT_MIN", "0"))
        identf = const.tile([128, 128], f32)
        identb = const.tile([128, 128], bf16)
        iNeg = const.tile([128, 128], bf16)
        if min_mode < 1:
            make_identity(nc, identf[:])
            nc.vector.tensor_copy(identb[:], identf[:])
            # iNeg = -1e15 * I: mask enters scores as iNeg.T @ notE^T
            nc.vector.tensor_scalar(iNeg[:], identb[:], -NEG, -1.0,
                                    Alu.mult, Alu.mult)

        # resident tensors
        xall = const.tile([2 * C, NP, B], bf16)
        nc.sync.dma_start(out=xall[:], in_=xq[:, :, :])
        bbT_res = const.tile([C, NB], f32)
        adT_res = const.tile([128, 4, C], bf16)
        if os.environ.get("GAT_SKIP_ADT", "0") != "1" and min_mode < 2:
            for g in range(4):
                nc.gpsimd.dma_start(out=adT_res[:, g, :],
                                    in_=adT[g * 128:(g + 1) * 128, :])
        if os.environ.get("GAT_SKIP_BBT", "0") != "1" and min_mode < 2:
            bb_split = int(os.environ.get("GAT_BBT_SPLIT", "2"))
            step = NB // bb_split
            for s in range(bb_split):
                nc.gpsimd.dma_start(out=bbT_res[:, s * step:(s + 1) * step],
                                    in_=bbT[:, s * step:(s + 1) * step])

        # h^T (post-bias) + ones row, resident: rhs of the score matmuls
        hT_res = const.tile([C1, B, NB], bf16)
        if min_mode < 3:
            nc.vector.memset(hT_res[C:C1, :, :], 1.0)
        # notE^T resident: 1.0 where NOT an edge (j on partitions)
        ne_all = const.tile([128, NJT * NB], bf16)
        # a_src^T resident + per-batch waug (a_src^T rows, d row per batch)
        asT_res = const.tile([C, N], bf16)
        for s in range(8):
            nc.gpsimd.dma_start(out=asT_res[:, s * NB:(s + 1) * NB],
                                in_=asT[:, s * NB:(s + 1) * NB])
        waug_all = const.tile([C1, B, N], bf16)
        # gathered V' tiles for all (j-tile, b): [128, 65] slices
        v_all = const.tile([128, NJT, B * C1], bf16)
        # per-node attention-dst dot, all groups: d_all[:, b*4+g]
        d_all = const.tile([128, 16], f32)

        z1w = z_local.ap()[0:Z1].rearrange("(n b c) -> n b c", n=NB, b=B)

        # ---------------- Phase A: project local nodes ----------------
        for g in range(0 if skip_phase_a else n_groups):               # groups of 128 local nodes
            n0 = g * 128
            psum_h = ps_y.tile([2 * C, 64, B], f32, tag="psy")
            wg = pa_sb.tile([128, 64, 2 * C], bf16, tag="wg")
            nc.sync.dma_start(out=wg[:], in_=wb[:, g * 64:(g + 1) * 64, :])
            for t in range(64):                 # node pairs (2t, 2t+1)
                tg = g * 64 + t
                x_pair = pa_x.tile([128, B], bf16, tag="xp")
                nc.vector.tensor_copy(x_pair[:], xall[:, tg, :])
                w_pair = pa_w.tile([128, 2 * C], bf16, tag="wp")
                if t % 2 == 0 and os.environ.get("GAT_W_ACT", "1") == "1":
                    nc.scalar.activation(w_pair[:], wg[:, t, :], Act.Copy)
                else:
                    nc.vector.tensor_copy(w_pair[:], wg[:, t, :])
                if pa_level < 2:
                    continue
                # block-diag pair weights: out rows 0:64 even node, 64:128 odd
                nc.tensor.matmul(psum_h[:, t, :], w_pair[:], x_pair[:],
                                 start=True, stop=True)
            # psum_h[(e,o), t, b] = h[b, n0+2t+e, o] (pre-bias)
            if pa_level < 3:
                continue
            for b in range(B):
                for e in range(2):
                    nc.vector.tensor_add(
                        hT_res[0:C, b, n0 + e:n0 + 128:2],
                        psum_h[e * C:(e + 1) * C, :, b],
                        bbT_res[:, n0 + e:n0 + 128:2])

            h_nat = pa_sb.tile([128, B, C1], bf16, tag="hnat")
            nc.vector.memset(h_nat[:, :, C:C1], 1.0)
            dtmp = pa_sb.tile([128, C], f32, tag="dtmp")
            for b in range(B):
                if pa_level < 4:
                    continue
                pst = ps_t.tile([128, C], bf16, tag="pst")
                nc.tensor.transpose(pst[:],
                                    hT_res[0:C, b, n0:n0 + 128],
                                    identb[0:C, 0:C])
                nc.vector.tensor_copy(h_nat[:, b, 0:C], pst[:])
                if pa_level < 5:
                    continue
                nc.vector.tensor_mul(dtmp[:], h_nat[:, b, 0:C],
                                     adT_res[:, g, :])
                nc.vector.reduce_sum(d_all[:, b * 4 + g:b * 4 + g + 1],
                                     dtmp[:], axis=mybir.AxisListType.X)
            if pa_level < 6:
                continue
            if os.environ.get("GAT_Z_GPSIMD", "1") == "1":
                nc.gpsimd.dma_start(out=z1w[n0:n0 + 128, :, :], in_=h_nat[:])
            else:
                nc.sync.dma_start(out=z1w[n0:n0 + 128, :, :], in_=h_nat[:])

        # d values -> z region 2 as [b][n] rows (transpose on PE)
        if skip_phase_a:
            nc.vector.memset(d_all[:], 0.0)
        if pa_level >= 7:
            pst2 = ps_t.tile([16, 128], f32, tag="pst")
            nc.tensor.transpose(pst2[:], d_all[:], identf[:])
            dT_sb = pa_sb.tile([16, 128], bf16, tag="dts")
            nc.vector.tensor_copy(dT_sb[:], pst2[:])
            for b in range(B):
                z2v = z_local.ap()[Z1 + b * NB:Z1 + (b + 1) * NB].rearrange(
                    "(g n) -> g n", g=4)
                nc.gpsimd.dma_start(out=z2v, in_=dT_sb[b * 4:(b + 1) * 4, :])

        # ---------------- Phase C: adjacency mask (independent) ----------------
        # notE^T[j, i] = 1.0 if NOT ((0 < dist[i,j] < 0.5)) else 0.0
        # (diagonal pre-patched to 0.25 on host => edge)
        if skip_mask:
            nc.vector.memset(ne_all[:], 0.0)
        for jt in range(0 if skip_mask else NJT):
            j0 = jt * JT
            dt_ = mk_sb.tile([128, NB], f32, tag="dt")
            nc.sync.dma_start(out=dt_[:], in_=dbT[j0:j0 + JT, :])
            t1 = mk_sb.tile([128, NB], f32, tag="t1")
            nc.vector.tensor_scalar(t1[:], dt_[:], 0.5, None, Alu.is_ge)
            nc.vector.scalar_tensor_tensor(
                out=ne_all[:, jt * NB:(jt + 1) * NB],
                in0=dt_[:], scalar=0.0, in1=t1[:],
                op0=Alu.is_le, op1=Alu.max)

        # ---------------- Phase B: all-gather projected features ----------------
        if not skip_coll:
            nc.gpsimd.collective_compute(
                "AllGather",
                mybir.AluOpType.bypass,
                replica_groups=[list(range(R))],
                ins=[z_local.ap().opt()],
                outs=[z_full.ap().opt()],
            )

        # V' tiles for all j-tiles/batches (one contiguous DMA per j-tile)
        if skip_vall:
            nc.vector.memset(v_all[:], 0.0)
        for r in range(0 if skip_vall else R):
            z1f = z_full.ap()[r, 0:Z1].rearrange("(jl p b c) -> jl p (b c)",
                                                 jl=4, p=128, b=B)
            nc.sync.dma_start(
                out=v_all[:, r * 4:(r + 1) * 4, :].rearrange(
                    "p jl w -> jl p w") if False else
                v_all[:, r * 4:(r + 1) * 4, :],
                in_=z1f.rearrange("jl p w -> p jl w"))

        # ---------------- Phase D: scores / softmax / aggregation ----------------
        for b in range(B):
            nc.vector.tensor_copy(waug_all[0:C, b, :], asT_res[:])
            for r in range(0 if skip_coll else R):
                zd = z_full.ap()[r, Z1 + b * NB:Z1 + (b + 1) * NB].rearrange(
                    "(o n) -> o n", o=1)
                nc.sync.dma_start(out=waug_all[C:C1, b, r * NB:(r + 1) * NB],
                                  in_=zd)

        for b in range(n_batch if not skip_main else 0):
            waug = waug_all[:, b, :]

            psum_y = ps_y.tile([C1, NB], f32, tag="psy")
            # software pipeline: agg matmuls for group g-1 issue after the
            # scores of group g so the PE queue never head-blocks on the
            # DVE/ACT elementwise chain. ACT runs ONLY Exp (a single
            # activation-table load); leaky runs on DVE.
            p_hist = {}

            def _agg(gg):
                pp = p_hist.pop(gg)
                for q in range(GJ):
                    jt = gg * GJ + q
                    nc.tensor.matmul(psum_y[:],
                                     v_all[:, jt, b * C1:(b + 1) * C1],
                                     pp[:, q * NB:(q + 1) * NB],
                                     start=(jt == 0), stop=(jt == NJT - 1))

            for g in range(NG):
                psum_s = ps_s.tile([128, GJ * NB], f32, tag="pss")
                for q in range(GJ):
                    jt = g * GJ + q
                    if not d_noscores:
                        nc.tensor.matmul(psum_s[:, q * NB:(q + 1) * NB],
                                         waug[:, jt * JT:(jt + 1) * JT],
                                         hT_res[:, b, :],
                                         start=True, stop=d_nomask)
                    if not d_nomask:
                        nc.tensor.matmul(psum_s[:, q * NB:(q + 1) * NB],
                                         iNeg[:],
                                         ne_all[:, jt * NB:(jt + 1) * NB],
                                         start=d_noscores, stop=True)
                if g >= 1 and not d_noagg:
                    _agg(g - 1)
                # p = exp(leaky_relu(v)) = max(exp(v), exp(alpha*v)):
                # cols 0:F via double-exp (ACT) + max (DVE);
                # cols F:2NB via copy+leaky (DVE) + exp (ACT).
                # ACT only ever runs Exp => one activation-table load.
                if d_noew:
                    p = ew_sb.tile([128, GJ * NB], bf16, tag="pt")
                    nc.vector.tensor_copy(p[:], psum_s[:])
                    p_hist[g] = p
                else:
                    F = ew_split
                    p = ew_sb.tile([128, GJ * NB], bf16, tag="pt")
                    if F > 0 and g % 2 == 1 and os.environ.get(
                            "GAT_EW_BF16ALT", "1") == "1":
                        # bf16-staged variant: on HW, ScalarE reads bf16 at
                        # 2x; the DVE copy rides its phase-D slack.
                        tcp = ew_sb.tile([128, F], bf16, tag="tcp")
                        nc.vector.tensor_copy(tcp[:], psum_s[:, 0:F])
                        e1 = ew_sb.tile([128, F], bf16, tag="e1")
                        nc.scalar.activation(e1[:], tcp[:], Act.Exp)
                        e2 = ew_sb.tile([128, F], bf16, tag="e2")
                        nc.scalar.activation(e2[:], tcp[:], Act.Exp,
                                             scale=ALPHA)
                        nc.vector.tensor_tensor(
                            out=p[:, 0:F], in0=e1[:], in1=e2[:], op=Alu.max)
                    elif F > 0:
                        e1 = ew_sb.tile([128, F], bf16, tag="e1")
                        nc.scalar.activation(e1[:], psum_s[:, 0:F], Act.Exp)
                        e2 = ew_sb.tile([128, F], bf16, tag="e2")
                        nc.scalar.activation(e2[:], psum_s[:, 0:F], Act.Exp,
                                             scale=ALPHA)
                        nc.vector.tensor_tensor(
                            out=p[:, 0:F], in0=e1[:], in1=e2[:], op=Alu.max)
                    if F < GJ * NB:
                        W2 = GJ * NB - F
                        tcp = ew_sb.tile([128, W2], bf16, tag="tcp")
                        nc.vector.tensor_copy(tcp[:], psum_s[:, F:GJ * NB])
                        u = ew_sb.tile([128, W2], bf16, tag="ut")
                        nc.vector.scalar_tensor_tensor(
                            out=u[:], in0=tcp[:], scalar=ALPHA,
                            in1=tcp[:], op0=Alu.mult, op1=Alu.max)
                        nc.scalar.activation(p[:, F:GJ * NB], u[:], Act.Exp)
                    p_hist[g] = p
            if not d_noagg:
                _agg(NG - 1)
            else:
                pp = p_hist[max(p_hist)]
                nc.tensor.matmul(psum_y[:], v_all[:, 0, b * C1:(b + 1) * C1],
                                 pp[:, 0:NB], start=True, stop=True)

            # ---------------- normalize + write out ----------------
            y_sb = ep_sb.tile([C1, NB], f32, tag="ysb")
            nc.vector.tensor_copy(y_sb[:], psum_y[:])
            for g4 in range(4):
                pst = ps_t.tile([128, C1], f32, tag="pst")
                nc.tensor.transpose(pst[:],
                                    y_sb[:, g4 * 128:(g4 + 1) * 128],
                                    identf[0:C1, 0:C1])
                rec = ep_sb.tile([128, 1], f32, tag="rec")
                nc.vector.reciprocal(rec[:], pst[:, C:C1])
                y_out = ep_sb.tile([128, C], f32, tag="yout")
                nc.vector.tensor_scalar(y_out[:], pst[:, 0:C], rec[:],
                                        None, Alu.mult)
                nc.sync.dma_start(out=out[b, g4 * 128:(g4 + 1) * 128, :],
                                  in_=y_out[:])


def _get_nc():
    if "nc" not in _CACHE:
        _CACHE["nc"] = _build()
    return _CACHE["nc"]


def _pack_w(w_blk):
    """[NB, C, C] -> zero-padded block-diagonal pair weights [128, NP, 128]."""
    wpad = np.zeros((2 * C, NP, 2 * C), dtype=BF16)
    wpad[:C, :, :C] = w_blk[0::2].transpose(1, 0, 2)
    wpad[C:, :, C:] = w_blk[1::2].transpose(1, 0, 2)
    return wpad


def _make_in_maps(inputs):
    x = np.asarray(inputs["x"], dtype=np.float32)
    dist_mat = np.asarray(inputs["dist_mat"], dtype=np.float32)
    proj_w = np.asarray(inputs["proj_w"], dtype=np.float32)
    proj_b = np.asarray(inputs["proj_b"], dtype=np.float32)
    a_w = np.asarray(inputs["a_w"], dtype=np.float32)

    asT = np.ascontiguousarray(a_w[:, :C].T).astype(BF16)  # [64, 4096]
    in_maps = []
    idx = np.arange(NB)
    for k in range(R):
        blk = slice(k * NB, (k + 1) * NB)
        dbT_k = np.ascontiguousarray(dist_mat[blk, :].T)   # [4096, 512]
        dbT_k[k * NB + idx, idx] = 0.25  # force diagonal -> edge (adj |= eye)
        # x packed: [c2, pair, b] with c2 = (n%2)*64 + c
        xq_k = np.ascontiguousarray(
            x[:, blk, :].reshape(B, NP, 2, C).transpose(2, 3, 1, 0)
            .reshape(2 * C, NP, B)).astype(BF16)
        in_maps.append({
            "wb": _pack_w(proj_w[blk]),
            "xq": xq_k,
            "bbT": np.ascontiguousarray(proj_b[blk].T),
            "adT": a_w[blk, C:].astype(BF16),
            "dbT": dbT_k,
            "asT": asT,
        })
    return in_maps


def kernel(x, dist_mat, proj_w, proj_b, a_w):
    from concourse.bass_utils import run_bass_kernel_spmd

    nc = _get_nc()
    in_maps = _make_in_maps({"x": x, "dist_mat": dist_mat, "proj_w": proj_w,
                             "proj_b": proj_b, "a_w": a_w})
    last_err = None
    for _attempt in range(3):
        try:
            res = run_bass_kernel_spmd(nc, in_maps, core_ids=list(range(R)))
            outs = [res.results[k]["out"] for k in range(R)]
            return np.concatenate(outs, axis=1).astype(np.float32)
        except Exception as e:  # transient runtime/device errors: retry
            last_err = e
    raise last_err
